# revision 8
# baseline (speedup 1.0000x reference)
"""GCN encoder (5-layer GCNConv + BN + relu, global_add_pool) on 8 Trainium2 cores.

Strategy (graph/data parallel, per sharding hint):
  - Nodes are partitioned across the 8 cores (bin-packed into 128-node blocks so
    every block has a bounded number of incident edges from each half of the
    node set -> uniform SPMD program, per-core variation lives only in data).
  - Per layer: dense matmul h@W locally, AllGather the (bf16) node-feature
    table, then each core gathers source rows for its incident edges with
    dma_gather and segment-sums them into its destination blocks via one-hot
    matmuls on the PE (norm folded into the one-hot selection matrix).
  - BN statistics: per-core partial sums via activation accum_out on the
    transposed tiles, AllReduce'd across cores; BN apply is fused into a
    single per-partition scale/bias (+relu) activation op.
  - Pooling: one-hot matmul against graph ids, AllReduce of the [256,128]
    partial, so every core holds the full pooled output.
"""
import sys

sys.path.insert(0, "/opt/trn_rl_repo")

import numpy as np

import concourse.bacc as bacc
import concourse.tile as tile
import concourse.mybir as mybir
from concourse import bass_utils
from concourse.masks import make_identity

# ---------------------------------------------------------------- constants
P = 128
NCORES = 8
N = 50000
EMB = 128
L = 5
G = 256
BN_EPS = 1e-5

HALF = 25000          # node-id split of sources into two int16-indexable halves
NB = 50               # dest blocks per core
CT = 9                # edge tiles per (block, stream)
ROWS = NB * P         # 6400 padded node slots per core
TBL = ROWS * NCORES   # 51200 table rows after AllGather
NTILES = NB * CT      # 450 edge tiles per stream per core
CHUNK_BLK = 5
CHUNK_TILES = CT * CHUNK_BLK    # 45 tiles per gather chunk
NCHUNKS = NB // CHUNK_BLK       # 10 chunks per stream per layer
NIDX = CHUNK_TILES * P          # 5760 indices per gather

F32 = mybir.dt.float32
BF16 = mybir.dt.bfloat16
I16 = mybir.dt.int16
AF = mybir.ActivationFunctionType
ALU = mybir.AluOpType

_CACHE = {}


# ---------------------------------------------------------------- device program
def _build_program():
    if "nc" in _CACHE:
        return _CACHE["nc"]

    nc = bacc.Bacc("TRN2", target_bir_lowering=False, debug=False,
                   num_devices=NCORES)

    # per-core inputs
    xT_in = nc.dram_tensor("xT_in", [P, ROWS], F32, kind="ExternalInput")
    w_in = nc.dram_tensor("w_in", [P, L * EMB], F32, kind="ExternalInput")
    gb_in = nc.dram_tensor("gb_in", [P, 2 * L], F32, kind="ExternalInput")
    idx_in = [nc.dram_tensor(f"idx{s}_in", [P, NTILES * 8], I16, kind="ExternalInput")
              for s in range(2)]
    seg_in = [nc.dram_tensor(f"seg{s}_in", [P, NTILES], F32, kind="ExternalInput")
              for s in range(2)]
    nrm_in = [nc.dram_tensor(f"nrm{s}_in", [P, NTILES], F32, kind="ExternalInput")
              for s in range(2)]
    pseg_in = [nc.dram_tensor(f"pseg{g}_in", [P, NB], F32, kind="ExternalInput")
               for g in range(2)]

    h_out = nc.dram_tensor("h_out", [ROWS, EMB], F32, kind="ExternalOutput")
    xpool_out = nc.dram_tensor("xpool_out", [G, EMB], F32, kind="ExternalOutput")

    rg = [list(range(NCORES))]

    with tile.TileContext(nc) as tc:
        with tc.tile_pool(name="persist", bufs=1) as pers, \
             tc.tile_pool(name="work", bufs=3) as wk, \
             tc.tile_pool(name="gpool", bufs=2) as gp, \
             tc.tile_pool(name="spool", bufs=4) as sp, \
             tc.tile_pool(name="dense_ps", bufs=2, space="PSUM") as dense_ps, \
             tc.tile_pool(name="edge_ps", bufs=2, space="PSUM") as edge_ps, \
             tc.tile_pool(name="trans_ps", bufs=2, space="PSUM") as trans_ps, \
             tc.tile_pool(name="dram", bufs=1, space="DRAM") as dr:

            # ---------- persistent state / constants
            hT = pers.tile([P, ROWS], F32)          # [feat, nodeslot] current h (transposed)
            h_raw = pers.tile([P, NB * P], F32)     # [nodeslot-in-block, feat] per block
            w_sb = pers.tile([P, L * EMB], F32)
            gb_sb = pers.tile([P, 2 * L], F32)
            iota_bf = pers.tile([P, P], BF16)
            iota_f = pers.tile([P, P], F32)
            ident = pers.tile([P, P], F32)
            idx_sb = [pers.tile([P, NTILES * 8], I16, name=f"idxsb{s}") for s in range(2)]
            seg_sb = [pers.tile([P, NTILES], F32, name=f"segsb{s}") for s in range(2)]
            nrm_sb = [pers.tile([P, NTILES], F32, name=f"nrmsb{s}") for s in range(2)]
            pseg_sb = [pers.tile([P, NB], F32, name=f"psegsb{g}") for g in range(2)]
            stat_sum = pers.tile([P, NB], F32)
            stat_sq = pers.tile([P, NB], F32)
            eps_t = pers.tile([P, 1], F32)
            nc.gpsimd.memset(eps_t[:], BN_EPS)

            nc.sync.dma_start(hT[:], xT_in[:])
            nc.sync.dma_start(w_sb[:], w_in[:])
            nc.sync.dma_start(gb_sb[:], gb_in[:])
            for s in range(2):
                nc.sync.dma_start(idx_sb[s][:], idx_in[s][:])
                nc.sync.dma_start(seg_sb[s][:], seg_in[s][:])
                nc.sync.dma_start(nrm_sb[s][:], nrm_in[s][:])
            for g in range(2):
                nc.sync.dma_start(pseg_sb[g][:], pseg_in[g][:])
            nc.gpsimd.iota(iota_bf[:], pattern=[[1, P]], base=0, channel_multiplier=0,
                           allow_small_or_imprecise_dtypes=True)
            nc.gpsimd.iota(iota_f[:], pattern=[[1, P]], base=0, channel_multiplier=0,
                           allow_small_or_imprecise_dtypes=True)
            make_identity(nc, ident[:])

            # DRAM buffers (bounce + table); Shared tensors are write-once, so
            # collective outputs get a fresh tensor per layer.
            hw_local = dr.tile([ROWS, EMB], BF16)
            st_in = dr.tile([P, 2], F32)
            xp_local = dr.tile([G, EMB], F32)
            xp_red = dr.tile([G, EMB], F32, addr_space="Shared")

            for li in range(L):
                hw_table = dr.tile([TBL, EMB], BF16, addr_space="Shared",
                                   name=f"hw_table{li}")
                st_out = dr.tile([P, 2], F32, addr_space="Shared",
                                 name=f"st_out{li}")
                # ---------- dense: hw = h @ W[li]  (bf16 table rows)
                for b in range(NB):
                    cs = slice(b * P, (b + 1) * P)
                    mm = dense_ps.tile([P, EMB], F32, tag="dps")
                    nc.tensor.matmul(mm[:], lhsT=hT[:, cs],
                                     rhs=w_sb[:, li * EMB:(li + 1) * EMB],
                                     start=True, stop=True)
                    hw_sb = wk.tile([P, EMB], BF16, tag="hwsb")
                    nc.scalar.activation(hw_sb[:], mm[:], AF.Copy)
                    nc.sync.dma_start(hw_local[cs, :], hw_sb[:])

                # ---------- allgather node features
                nc.gpsimd.collective_compute(
                    "AllGather", ALU.bypass, replica_groups=rg,
                    ins=[hw_local[:].opt()], outs=[hw_table[:].opt()])

                # ---------- edge message passing (segment sum via one-hot matmul)
                for c in range(NCHUNKS):
                    gbuf = []
                    for s in range(2):
                        gb_t = gp.tile([P, CHUNK_TILES, P], BF16, tag=f"g{s}",
                                       name=f"gbuf{s}")
                        view = (hw_table[0:TBL // 2, :] if s == 0
                                else hw_table[TBL // 2:TBL, :])
                        nc.gpsimd.dma_gather(
                            out_ap=gb_t[:], in_ap=view,
                            idxs_ap=idx_sb[s][:, c * (NIDX // 16):(c + 1) * (NIDX // 16)],
                            num_idxs=NIDX, num_idxs_reg=NIDX, elem_size=EMB,
                            single_packet=False)
                        gbuf.append(gb_t)
                    for bb in range(CHUNK_BLK):
                        b = c * CHUNK_BLK + bb
                        eps = edge_ps.tile([P, EMB], F32, tag="eps")
                        for s in range(2):
                            for r in range(CT):
                                t = b * CT + r          # tile idx within stream
                                tc_ = bb * CT + r       # tile idx within chunk
                                S = sp.tile([P, P], BF16, tag="S")
                                nc.vector.tensor_scalar(
                                    out=S[:], in0=iota_bf[:],
                                    scalar1=seg_sb[s][:, t:t + 1],
                                    scalar2=nrm_sb[s][:, t:t + 1],
                                    op0=ALU.is_equal, op1=ALU.mult)
                                nc.tensor.matmul(
                                    eps[:], lhsT=S[:], rhs=gbuf[s][:, tc_, :],
                                    start=(s == 0 and r == 0),
                                    stop=(s == 1 and r == CT - 1))
                        cs = slice(b * P, (b + 1) * P)
                        nc.scalar.activation(h_raw[:, cs], eps[:], AF.Copy)
                        # transpose + BN stats
                        tp = trans_ps.tile([P, P], F32, tag="tps")
                        nc.tensor.transpose(tp[:], h_raw[:, cs], ident[:])
                        nc.scalar.activation(hT[:, cs], tp[:], AF.Copy,
                                             accum_out=stat_sum[:, b:b + 1])
                        junk = wk.tile([P, P], F32, tag="junk")
                        nc.scalar.activation(junk[:], tp[:], AF.Square,
                                             accum_out=stat_sq[:, b:b + 1])

                # ---------- BN stats allreduce
                stats2 = wk.tile([P, 2], F32, tag="misc2")
                nc.vector.reduce_sum(out=stats2[:, 0:1], in_=stat_sum[:],
                                     axis=mybir.AxisListType.X)
                nc.vector.reduce_sum(out=stats2[:, 1:2], in_=stat_sq[:],
                                     axis=mybir.AxisListType.X)
                nc.sync.dma_start(st_in[:], stats2[:])
                nc.gpsimd.collective_compute(
                    "AllReduce", ALU.add, replica_groups=rg,
                    ins=[st_in[:].opt()], outs=[st_out[:].opt()])
                st_back = wk.tile([P, 2], F32, tag="misc2")
                nc.sync.dma_start(st_back[:], st_out[:])

                # scale = gamma * rsqrt(var+eps); shift = beta - mean*scale
                vec = wk.tile([P, 8], F32, tag="vec")
                nc.vector.tensor_scalar(out=vec[:, 0:1], in0=st_back[:, 0:1],
                                        scalar1=1.0 / N, scalar2=None, op0=ALU.mult)  # mean
                nc.vector.tensor_scalar(out=vec[:, 1:2], in0=st_back[:, 1:2],
                                        scalar1=1.0 / N, scalar2=None, op0=ALU.mult)  # E[x^2]
                nc.vector.tensor_tensor(out=vec[:, 2:3], in0=vec[:, 0:1],
                                        in1=vec[:, 0:1], op=ALU.mult)                 # mean^2
                nc.vector.tensor_tensor(out=vec[:, 3:4], in0=vec[:, 1:2],
                                        in1=vec[:, 2:3], op=ALU.subtract)             # var
                nc.scalar.activation(vec[:, 4:5], vec[:, 3:4], AF.Sqrt, bias=eps_t[:])  # sd
                nc.vector.reciprocal(vec[:, 5:6], vec[:, 4:5])                        # rstd
                nc.vector.tensor_tensor(out=vec[:, 6:7], in0=vec[:, 5:6],
                                        in1=gb_sb[:, li:li + 1], op=ALU.mult)         # scale
                nc.vector.tensor_tensor(out=vec[:, 7:8], in0=vec[:, 0:1],
                                        in1=vec[:, 6:7], op=ALU.mult)                 # mean*scale
                shift = wk.tile([P, 1], F32, tag="shift")
                nc.vector.tensor_tensor(out=shift[:], in0=gb_sb[:, L + li:L + li + 1],
                                        in1=vec[:, 7:8], op=ALU.subtract)             # shift

                if li < L - 1:
                    # BN + relu applied in place on hT (per-partition scale/bias)
                    for b in range(NB):
                        cs = slice(b * P, (b + 1) * P)
                        nc.scalar.activation(hT[:, cs], hT[:, cs], AF.Relu,
                                             bias=shift[:], scale=vec[:, 6:7])
                else:
                    # last layer: apply BN in [node, feat] layout via broadcast tiles
                    sc_ps = trans_ps.tile([P, P], F32, tag="tps")
                    nc.tensor.transpose(sc_ps[:], vec[:, 6:7].to_broadcast([P, P]),
                                        ident[:])
                    scaleT = pers.tile([P, P], F32)
                    nc.scalar.activation(scaleT[:], sc_ps[:], AF.Copy)
                    sh_ps = trans_ps.tile([P, P], F32, tag="tps")
                    nc.tensor.transpose(sh_ps[:], shift[:].to_broadcast([P, P]),
                                        ident[:])
                    shiftT = pers.tile([P, P], F32)
                    nc.scalar.activation(shiftT[:], sh_ps[:], AF.Copy)

                    pool_ps = [dense_ps.tile([P, EMB], F32, tag="dps",
                                             name=f"poolps{g}") for g in range(2)]
                    for b in range(NB):
                        cs = slice(b * P, (b + 1) * P)
                        nc.vector.tensor_tensor(out=h_raw[:, cs], in0=h_raw[:, cs],
                                                in1=scaleT[:], op=ALU.mult)
                        nc.vector.tensor_tensor(out=h_raw[:, cs], in0=h_raw[:, cs],
                                                in1=shiftT[:], op=ALU.add)
                        nc.sync.dma_start(h_out[b * P:(b + 1) * P, :], h_raw[:, cs])
                        for g in range(2):
                            Sp = sp.tile([P, P], F32, tag="SP")
                            nc.vector.tensor_scalar(
                                out=Sp[:], in0=iota_f[:],
                                scalar1=pseg_sb[g][:, b:b + 1], scalar2=None,
                                op0=ALU.is_equal)
                            nc.tensor.matmul(pool_ps[g][:], lhsT=Sp[:],
                                             rhs=h_raw[:, cs],
                                             start=(b == 0), stop=(b == NB - 1))
                    for g in range(2):
                        xp_sb = wk.tile([P, EMB], F32, tag="xpsb")
                        nc.scalar.activation(xp_sb[:], pool_ps[g][:], AF.Copy)
                        nc.sync.dma_start(xp_local[g * P:(g + 1) * P, :], xp_sb[:])
                    nc.gpsimd.collective_compute(
                        "AllReduce", ALU.add, replica_groups=rg,
                        ins=[xp_local[:].opt()], outs=[xp_red[:].opt()])
                    for g in range(2):
                        xp_back = wk.tile([P, EMB], F32, tag="xpsb")
                        nc.sync.dma_start(xp_back[:], xp_red[g * P:(g + 1) * P, :])
                        nc.sync.dma_start(xpool_out[g * P:(g + 1) * P, :], xp_back[:])

    nc.compile()
    _CACHE["nc"] = nc
    return nc


# ---------------------------------------------------------------- host preprocessing
def _preprocess(batch, x, edge_index, edge_weight):
    batch = np.asarray(batch).astype(np.int64)
    x = np.asarray(x).astype(np.float32)
    ei = np.asarray(edge_index).astype(np.int64)
    ew = np.asarray(edge_weight).astype(np.float32)

    row = np.concatenate([ei[0], np.arange(N, dtype=np.int64)])
    col = np.concatenate([ei[1], np.arange(N, dtype=np.int64)])
    w = np.concatenate([ew, np.ones(N, np.float32)])

    deg = np.bincount(col, weights=w.astype(np.float64), minlength=N).astype(np.float32)
    dinv = np.where(deg > 0, 1.0 / np.sqrt(deg), 0.0).astype(np.float32)
    norm = (dinv[row] * w * dinv[col]).astype(np.float32)

    # ---- node -> (core, block, slot) assignment (two-resource bin packing)
    d0 = np.bincount(col[row < HALF], minlength=N)
    d1 = np.bincount(col[row >= HALF], minlength=N)
    node_core = np.empty(N, np.int32)
    node_block = np.empty(N, np.int32)   # block within core
    node_slot = np.empty(N, np.int32)

    nbh = NB * 4  # blocks per half
    for h in range(2):
        nodes = np.arange(h * HALF, (h + 1) * HALF)
        order = nodes[np.argsort(-(d0[nodes] + d1[nodes]), kind="stable")]
        b0 = np.zeros(nbh)
        b1 = np.zeros(nbh)
        bc = np.zeros(nbh, np.int64)
        gb = np.empty(HALF, np.int64)  # global block within half for each node (by order pos)
        for i, v in enumerate(order):
            ok = np.flatnonzero(bc < P)
            j = ok[np.argmin(np.maximum(b0[ok] + d0[v], b1[ok] + d1[v])
                             + 0.001 * (b0[ok] + b1[ok]))]
            gb[i] = j
            node_core[v] = h * 4 + j // NB
            node_block[v] = j % NB
            node_slot[v] = bc[j]
            b0[j] += d0[v]
            b1[j] += d1[v]
            bc[j] += 1
        assert b0.max() <= CT * P and b1.max() <= CT * P, \
            f"block edge cap exceeded: {b0.max()}, {b1.max()} > {CT * P}"

    table_pos = node_core.astype(np.int64) * ROWS + node_block * P + node_slot

    # ---- per-core edge arrays
    src_half = (row >= HALF).astype(np.int64)           # stream id
    dst_core = node_core[col]
    dst_block = node_block[col]
    dst_slot = node_slot[col]

    in_maps = []
    for k in range(NCORES):
        m = {}
        sel_core = dst_core == k
        for s in range(2):
            idx_arr = np.zeros(NTILES * P, np.int16)
            seg_arr = np.zeros(NTILES * P, np.float32)
            nrm_arr = np.zeros(NTILES * P, np.float32)
            sel = np.flatnonzero(sel_core & (src_half == s))
            blk = dst_block[sel]
            order = np.argsort(blk, kind="stable")
            sel = sel[order]
            blk = blk[order]
            counts = np.bincount(blk, minlength=NB)
            starts = np.zeros(NB, np.int64)
            starts[1:] = np.cumsum(counts)[:-1]
            rank = np.arange(len(sel)) - starts[blk]
            pos = blk.astype(np.int64) * (CT * P) + rank
            assert rank.max(initial=0) < CT * P
            idx_arr[pos] = (table_pos[row[sel]] - s * (TBL // 2)).astype(np.int16)
            seg_arr[pos] = dst_slot[sel]
            nrm_arr[pos] = norm[sel]
            m[f"idx{s}_in"] = np.ascontiguousarray(
                np.tile(idx_arr.reshape(-1, 16).T, (8, 1))).astype(np.int16)
            m[f"seg{s}_in"] = np.ascontiguousarray(seg_arr.reshape(NTILES, P).T)
            m[f"nrm{s}_in"] = np.ascontiguousarray(nrm_arr.reshape(NTILES, P).T)

        # node data
        xt = np.zeros((ROWS, EMB), np.float32)
        ps0 = -np.ones((NB, P), np.float32)
        ps1 = -np.ones((NB, P), np.float32)
        mynodes = np.flatnonzero(node_core == k)
        lpos = node_block[mynodes] * P + node_slot[mynodes]
        xt[lpos] = x[mynodes]
        ps0[node_block[mynodes], node_slot[mynodes]] = batch[mynodes]
        ps1[node_block[mynodes], node_slot[mynodes]] = batch[mynodes] - 128.0
        m["xT_in"] = np.ascontiguousarray(xt.T)
        m["pseg0_in"] = np.ascontiguousarray(ps0.T)
        m["pseg1_in"] = np.ascontiguousarray(ps1.T)
        in_maps.append(m)

    return in_maps, table_pos


def kernel(batch, x, edge_index, edge_weight, Ws, bs, gammas, betas):
    # bs is mathematically irrelevant: BN (training mode) removes any constant
    # per-feature offset added before it, so the conv bias cancels exactly.
    Ws = np.asarray(Ws).astype(np.float32)
    gammas = np.asarray(gammas).astype(np.float32)
    betas = np.asarray(betas).astype(np.float32)

    in_maps, table_pos = _preprocess(batch, x, edge_index, edge_weight)

    w_all = np.ascontiguousarray(np.concatenate([Ws[i] for i in range(L)], axis=1))
    gb_all = np.ascontiguousarray(np.concatenate([gammas, betas], axis=0).T)
    for m in in_maps:
        m["w_in"] = w_all
        m["gb_in"] = gb_all

    nc = _build_program()
    res = bass_utils.run_bass_kernel_spmd(nc, in_maps, core_ids=list(range(NCORES)))

    h_cat = np.concatenate([res.results[c]["h_out"] for c in range(NCORES)], axis=0)
    h = np.ascontiguousarray(h_cat[table_pos])
    xpool = np.asarray(res.results[0]["xpool_out"])
    return (xpool, h)
